# revision 44
# baseline (speedup 1.0000x reference)
"""Multi-head attention (B=2, S=2048, d_model=1024, 16 heads) on 8 TRN2 cores.

Sharding: core c -> batch b = c//4, head-group hg = c%4 (4 heads per core).
Each core computes its heads' attention + its partial w_o projection.
Host gathers: sums the w_o partials over the 4 cores of each batch (+b_o),
and normalizes/transposes the packed causal exp-score tiles into the full
[B, H, S, S] attention-weight tensor.

Device layout notes:
 - q/k/v are fed pre-transposed ([d_model, S]) so all DMAs are contiguous.
 - scores are computed transposed ([j, i]) so that the attn@V contraction
   (over j) can consume the exp tiles directly from SBUF, with a ones-column
   in V producing the softmax denominators in the same matmuls.
 - softmax skips max-subtraction: scores/8 are bounded (|x| < ~10), exp
   cannot overflow in f32, and exp(-1e9) underflows to exactly 0 like the
   reference's exp(x - max) does for masked entries.
 - causal masking is a 0/1 multiply on diagonal-straddling tiles only;
   tiles fully above the diagonal are neither computed nor written.
 - all big matmuls use float32r (4x fp32 throughput at moving-dim >= 256).
"""

import os
import numpy as np

import concourse.bass as bass
import concourse.bacc as bacc
import concourse.mybir as mybir
import concourse.tile as tile
from concourse.bass_utils import run_bass_kernel_spmd
from concourse.tile_sem_assignment import tick_to_sem


class TileContextSplitDrain(tile.TileContext):
    """TileContext whose kernel-tail drain carries at most one sync wait
    per instruction.

    The stock _drain_and_barrier attaches one sem-wait per live proc to a
    single Drain instruction; this walrus build caps every non-EVSEM
    instruction at one sync wait and fails codegen ("Too many sync wait
    commands"). Emit one standalone single-wait instruction per semaphore
    instead, then a wait-less drain.
    """

    def _drain_and_barrier(self, tick_clock, wait_clock):
        nc = self.nc
        gc = tick_clock.global_clock
        for proc, sem in sorted(self.sems.allocated().items()):
            tick = gc.peek_next(proc) - 1
            if tick > 0:
                nc.sync.wait_ge(sem, tick_to_sem(tick, proc))
        nc.sync.drain()
        nc.all_engine_barrier()
        popped = nc._tile_sem_poison_stack.pop()
        assert popped is self._sem_poison
        nc.clear_and_free_semaphores(list(self.sems.allocated().values()))
        nc.all_engine_barrier()

F32 = mybir.dt.float32
F32R = mybir.dt.float32r
AF = mybir.ActivationFunctionType

B, S, D, H, DK = 2, 2048, 1024, 16, 64
HPC = 4                      # heads per core
NCORES = 8
IB_N = 4                     # i-blocks of 512
TILE_OFF = [0, 4, 12, 24]    # running tile index per i-block
NTILES = 40                  # total lower-triangle [128,512] tiles per head

_nc_cache = None
last_results = None          # stashed BassKernelResults (for test harness)


def build_nc():
    nc = bacc.Bacc("TRN2", target_bir_lowering=False)

    qT = nc.declare_dram_parameter("qT", [D, S], F32R, isOutput=False)
    kT = nc.declare_dram_parameter("kT", [D, S], F32R, isOutput=False)
    vT = nc.declare_dram_parameter("vT", [D, S], F32R, isOutput=False)
    wq = nc.declare_dram_parameter("wq", [D, 256], F32R, isOutput=False)
    wk = nc.declare_dram_parameter("wk", [D, 256], F32R, isOutput=False)
    wv = nc.declare_dram_parameter("wv", [D, 256], F32R, isOutput=False)
    bq = nc.declare_dram_parameter("bq", [256], F32, isOutput=False)
    bk = nc.declare_dram_parameter("bk", [256], F32, isOutput=False)
    bv = nc.declare_dram_parameter("bv", [256], F32R, isOutput=False)
    wo = nc.declare_dram_parameter("wo", [64, HPC, D], F32R, isOutput=False)
    masks = nc.declare_dram_parameter("masks", [128, 896], F32R, isOutput=False)
    ones_mat = nc.declare_dram_parameter("ones_mat", [128, 128], F32R, isOutput=False)

    attn_o = nc.declare_dram_parameter("attn_o", [HPC, NTILES, 128, 512], F32R, isOutput=True)
    outp = nc.declare_dram_parameter("outp", [S, D], F32, isOutput=True)
    sums_o = nc.declare_dram_parameter("sums_o", [HPC, S], F32R, isOutput=True)

    with TileContextSplitDrain(nc) as tc:
        with tc.tile_pool(name="const", bufs=1) as const:
            masks_sb = const.tile([128, 896], F32R, tag="masks")
            nc.sync.dma_start(out=masks_sb, in_=masks[:, :])
            wo_sb = const.tile([64, HPC, D], F32R, tag="wo")
            nc.sync.dma_start(out=wo_sb[:, :, :], in_=wo[:, :, :])
            bq_sb = const.tile([128, 2], F32, tag="bq")
            nc.sync.dma_start(out=bq_sb, in_=bq[:].rearrange("(c p) -> p c", p=128))
            bk_sb = const.tile([128, 2], F32, tag="bk")
            nc.sync.dma_start(out=bk_sb, in_=bk[:].rearrange("(c p) -> p c", p=128))
            bv_row = const.tile([1, 256], F32R, tag="bv")
            nc.sync.dma_start(out=bv_row, in_=bv[:].rearrange("(a f) -> a f", a=1))
            # ones constants come from DRAM: memset cannot write f32r
            ones_r = const.tile([65, 64], F32R, tag="ones_r")  # row 64 used (base-64 matmul operand)
            nc.sync.dma_start(out=ones_r[64:65, :], in_=ones_mat[0:1, 0:64])
            ones_c = const.tile([1, 128], F32R, tag="ones_c")
            nc.sync.dma_start(out=ones_c, in_=ones_mat[0:1, :])

            # persistent activations
            qhT_sb = const.tile([128, 2, S], F32R, tag="qhT")   # [d' (2 chunks), s]
            khT_sb = const.tile([128, 2, S], F32R, tag="khT")
            vh1_sb = const.tile([128, 16, HPC, 65], F32R, tag="vh1")  # [s, jc, h, dk+1]
            # col 64 = 1.0 (softmax-denominator column); cols 0:64 written by V proj
            nc.sync.dma_start(
                out=vh1_sb[:, :, :, 64:65],
                in_=ones_mat[:, 0:64].rearrange("p (a b c) -> p a b c", a=16, b=HPC),
            )

            # ---------------- projections ----------------
            with tc.tile_pool(name="psproj", bufs=8, space="PSUM") as psproj, \
                 tc.tile_pool(name="xin", bufs=3) as xin, \
                 tc.tile_pool(name="wts", bufs=2) as wts:

                for xext, wext, dst, bias_sb in (
                    (qT, wq, qhT_sb, bq_sb),
                    (kT, wk, khT_sb, bk_sb),
                ):
                    w_sb = wts.tile([128, 8, 256], F32R, tag="w")
                    nc.sync.dma_start(out=w_sb, in_=wext[:, :].rearrange("(kc p) m -> p kc m", p=128))
                    # two half-tensor loads (4 d-chunks each) instead of 8 small ones
                    xh = []
                    for half in range(2):
                        xt = xin.tile([128, 4, S], F32R, tag="x", name=f"xt{half}")
                        eng = nc.sync if half == 0 else nc.scalar
                        eng.dma_start(
                            out=xt,
                            in_=xext[half * 512:(half + 1) * 512, :].rearrange(
                                "(kc p) s -> p kc s", p=128),
                        )
                        xh.append(xt)
                    ps = [psproj.tile([128, 512], F32, tag="pp", name=f"ps{i}") for i in range(8)]
                    for kc in range(8):
                        xt = xh[kc // 4]
                        for mc in range(2):
                            for sb4 in range(4):
                                nc.tensor.matmul(
                                    ps[mc * 4 + sb4][:, :],
                                    lhsT=w_sb[:, kc, mc * 128:(mc + 1) * 128],
                                    rhs=xt[:, kc % 4, sb4 * 512:(sb4 + 1) * 512],
                                    start=(kc == 0), stop=(kc == 7),
                                )
                    for mc in range(2):
                        for sb4 in range(4):
                            nc.vector.tensor_scalar_add(
                                out=dst[:, mc, sb4 * 512:(sb4 + 1) * 512],
                                in0=ps[mc * 4 + sb4][:, :],
                                scalar1=bias_sb[:, mc:mc + 1],
                            )

                # V projection: vh[s, d'] per s-chunk; two sweeps of 8 s-chunks
                # (PSUM can only hold 8 accumulators); vT stays resident.
                w_sb = wts.tile([128, 8, 256], F32R, tag="w")
                nc.sync.dma_start(out=w_sb, in_=wv[:, :].rearrange("(kc p) m -> p kc m", p=128))
                vh = []
                for half in range(2):
                    xt = xin.tile([128, 4, S], F32R, tag="x", name=f"vt{half}")
                    eng = nc.sync if half == 0 else nc.scalar
                    eng.dma_start(
                        out=xt,
                        in_=vT[half * 512:(half + 1) * 512, :].rearrange(
                            "(kc p) s -> p kc s", p=128),
                    )
                    vh.append(xt)
                for sweep in range(2):
                    psv = [psproj.tile([128, 256], F32, tag="pp", name=f"psv{i}") for i in range(8)]
                    for kc in range(8):
                        xt = vh[kc // 4]
                        for j8 in range(8):
                            jc = sweep * 8 + j8
                            nc.tensor.matmul(
                                psv[j8][:, :],
                                lhsT=xt[:, kc % 4, jc * 128:(jc + 1) * 128],
                                rhs=w_sb[:, kc, :],
                                start=(kc == 0), stop=False,
                            )
                    for j8 in range(8):
                        jc = sweep * 8 + j8
                        nc.tensor.matmul(   # + b_v (rank-1: ones x bv)
                            psv[j8][:, :],
                            lhsT=ones_c, rhs=bv_row,
                            start=False, stop=True,
                        )
                        nc.vector.tensor_copy(
                            out=vh1_sb[:, jc, :, 0:64],
                            in_=psv[j8][:, :].rearrange("p (h d) -> p h d", h=HPC),
                        )

            # ---------------- attention + output projection ----------------
            with tc.tile_pool(name="pss", bufs=2, space="PSUM") as pss, \
                 tc.tile_pool(name="pso", bufs=2, space="PSUM") as pso, \
                 tc.tile_pool(name="psb", bufs=1, space="PSUM") as psb, \
                 tc.tile_pool(name="psp", bufs=1, space="PSUM") as psp, \
                 tc.tile_pool(name="expp", bufs=2) as expp, \
                 tc.tile_pool(name="outT", bufs=6) as outTp, \
                 tc.tile_pool(name="rbp", bufs=2) as rbp, \
                 tc.tile_pool(name="rcp", bufs=2) as rcp, \
                 tc.tile_pool(name="outs", bufs=2) as outs:

                for ib in range(IB_N):
                    jn = 4 * (ib + 1)
                    ot_tiles = []
                    for h in range(HPC):
                        po = (h % 2) * 64
                        mc = h // 2
                        pso_t = pso.tile([65, 512], F32, tag="po")
                        expg = expp.tile([128, jn, 512], F32R, tag="e")
                        for jp in range(jn // 2):
                            ps2 = pss.tile([128, 2, 512], F32, tag="ps")
                            for half in range(2):
                                jc = 2 * jp + half
                                nc.tensor.matmul(
                                    ps2[:, half, :],
                                    lhsT=khT_sb[po:po + 64, mc, jc * 128:(jc + 1) * 128],
                                    rhs=qhT_sb[po:po + 64, mc, ib * 512:(ib + 1) * 512],
                                    start=True, stop=True,
                                )
                            # one exp over both score tiles (halves ACT op count)
                            nc.scalar.activation(out=expg[:, 2 * jp:2 * jp + 2, :],
                                                 in_=ps2[:, :, :], func=AF.Exp, scale=0.125)
                            for half in range(2):
                                jc = 2 * jp + half
                                et = expg[:, jc, :]
                                if jc >= 4 * ib:
                                    off = 384 - (jc - 4 * ib) * 128
                                    nc.vector.tensor_mul(out=et, in0=et,
                                                         in1=masks_sb[:, off:off + 512])
                                nc.tensor.matmul(
                                    pso_t[:, :],
                                    lhsT=vh1_sb[:, jc, h, :],
                                    rhs=et,
                                    start=(jc == 0), stop=(jc == jn - 1),
                                )
                        nc.sync.dma_start(
                            out=attn_o[h, TILE_OFF[ib]:TILE_OFF[ib] + jn].rearrange(
                                "t p f -> p t f"),
                            in_=expg[:, :, :],
                        )
                        # softmax denominators (row 64 of pso_t): reciprocal -> DRAM
                        rc_t = rcp.tile([65, 512], F32R, tag="rc")
                        with nc.allow_low_precision(reason="f32r recip feeds f32r matmuls"):
                            nc.vector.reciprocal(rc_t[64:65, :], pso_t[64:65, :])
                        nc.gpsimd.dma_start(
                            out=sums_o[h, ib * 512:(ib + 1) * 512].rearrange("(a f) -> a f", a=1),
                            in_=rc_t[64:65, :],
                        )
                        # broadcast 1/sum over the 64 dk partitions (rank-1 matmul)
                        psb_t = psb.tile([64, 512], F32, tag="pb")
                        nc.tensor.matmul(psb_t[:, :], lhsT=ones_r[64:65, :],
                                         rhs=rc_t[64:65, :], start=True, stop=True)
                        rb_t = rbp.tile([64, 512], F32, tag="rb")
                        nc.vector.tensor_copy(out=rb_t[:, :], in_=psb_t[:, :])
                        ot_t = outTp.tile([64, 512], F32R, tag="ot")
                        nc.vector.tensor_mul(out=ot_t[:, :], in0=pso_t[0:64, :], in1=rb_t[:, :])
                        ot_tiles.append(ot_t)

                    # w_o partial projection for this i-block
                    for ic in range(4):
                        i0 = ic * 128
                        ob_t = outs.tile([128, D], F32, tag="ob")
                        for nb in range(2):
                            pp_t = psp.tile([128, 512], F32, tag="ppj")
                            for h in range(HPC):
                                nc.tensor.matmul(
                                    pp_t[:, :],
                                    lhsT=ot_tiles[h][:, i0:i0 + 128],
                                    rhs=wo_sb[:, h, nb * 512:(nb + 1) * 512],
                                    start=(h == 0), stop=(h == HPC - 1),
                                )
                            nc.scalar.activation(out=ob_t[:, nb * 512:(nb + 1) * 512],
                                                 in_=pp_t[:, :], func=AF.Copy)
                        r0 = (ib * 4 + ic) * 128
                        nc.scalar.dma_start(out=outp[r0:r0 + 128, :], in_=ob_t)

    nc.compile()  # bacc: register allocation, DCE, event-sem legalization
    return nc


def _get_nc():
    global _nc_cache
    if _nc_cache is None:
        _nc_cache = build_nc()
    return _nc_cache


def make_core_inputs(q, k, v, w_q, b_q, w_k, b_k, w_v, b_v, w_o, c):
    b, hg = c // 4, c % 4
    sl = slice(hg * 256, hg * 256 + 256)
    masks = (np.arange(896)[None, :] >= (np.arange(128)[:, None] + 384)).astype(np.float32)
    return {
        "qT": np.ascontiguousarray(q[b].T),
        "kT": np.ascontiguousarray(k[b].T),
        "vT": np.ascontiguousarray(v[b].T),
        "wq": np.ascontiguousarray(w_q[sl, :].T),
        "wk": np.ascontiguousarray(w_k[sl, :].T),
        "wv": np.ascontiguousarray(w_v[sl, :].T),
        "bq": np.ascontiguousarray(b_q[sl]),
        "bk": np.ascontiguousarray(b_k[sl]),
        "bv": np.ascontiguousarray(b_v[sl]),
        "wo": np.ascontiguousarray(w_o[:, sl].T.reshape(HPC, 64, D).transpose(1, 0, 2)),
        "masks": masks,
        "ones_mat": np.ones((128, 128), np.float32),
    }


def gather(results, b_o):
    out = np.zeros((B, S, D), np.float32)
    attn = np.zeros((B, H, S, S), np.float32)
    for c in range(NCORES):
        r = results[c]
        b, hg = c // 4, c % 4
        out[b] += np.asarray(r["outp"])
        recips = np.asarray(r["sums_o"])     # [4, S] — 1/denominator
        ap = np.asarray(r["attn_o"])         # [4, 40, 128, 512]
        for h4 in range(HPC):
            h = hg * HPC + h4
            for ib in range(IB_N):
                jn = 4 * (ib + 1)
                blk = ap[h4, TILE_OFF[ib]:TILE_OFF[ib] + jn]      # [jc, j', i']
                s = recips[h4, ib * 512:(ib + 1) * 512]
                m = blk.transpose(2, 0, 1).reshape(512, jn * 128) * s[:, None]
                attn[b, h, ib * 512:(ib + 1) * 512, :jn * 128] = m
    out += np.asarray(b_o, np.float32)[None, None, :]
    return out, attn


def kernel(**inputs):
    global last_results
    q = np.asarray(inputs["q"], np.float32)
    k = np.asarray(inputs["k"], np.float32)
    v = np.asarray(inputs["v"], np.float32)
    w_q = np.asarray(inputs["w_q"], np.float32)
    b_q = np.asarray(inputs["b_q"], np.float32)
    w_k = np.asarray(inputs["w_k"], np.float32)
    b_k = np.asarray(inputs["b_k"], np.float32)
    w_v = np.asarray(inputs["w_v"], np.float32)
    b_v = np.asarray(inputs["b_v"], np.float32)
    w_o = np.asarray(inputs["w_o"], np.float32)
    b_o = np.asarray(inputs["b_o"], np.float32)

    nc = _get_nc()
    in_maps = [
        make_core_inputs(q, k, v, w_q, b_q, w_k, b_k, w_v, b_v, w_o, c)
        for c in range(NCORES)
    ]
    res = run_bass_kernel_spmd(nc, in_maps, list(range(NCORES)))
    last_results = res
    return gather(res.results, b_o)


# revision 45
# speedup vs baseline: 1.0903x; 1.0903x over previous
"""Multi-head attention (B=2, S=2048, d_model=1024, 16 heads) on 8 TRN2 cores.

Sharding: core c -> batch b = c//4, head-group hg = c%4 (4 heads per core).
Each core computes its heads' attention + its partial w_o projection.
Host gathers: sums the w_o partials over the 4 cores of each batch (+b_o),
and normalizes/transposes the packed causal exp-score tiles into the full
[B, H, S, S] attention-weight tensor.

Device layout notes:
 - q/k/v are fed pre-transposed ([d_model, S]) so all DMAs are contiguous.
 - scores are computed transposed ([j, i]) so that the attn@V contraction
   (over j) can consume the exp tiles directly from SBUF, with a ones-column
   in V producing the softmax denominators in the same matmuls.
 - softmax skips max-subtraction: scores/8 are bounded (|x| < ~10), exp
   cannot overflow in f32, and exp(-1e9) underflows to exactly 0 like the
   reference's exp(x - max) does for masked entries.
 - causal masking is a 0/1 multiply on diagonal-straddling tiles only;
   tiles fully above the diagonal are neither computed nor written.
 - all big matmuls use float32r (4x fp32 throughput at moving-dim >= 256).
"""

import os
import numpy as np

import concourse.bass as bass
import concourse.bacc as bacc
import concourse.mybir as mybir
import concourse.tile as tile
from concourse.bass_utils import run_bass_kernel_spmd
from concourse.tile_sem_assignment import tick_to_sem


class TileContextSplitDrain(tile.TileContext):
    """TileContext whose kernel-tail drain carries at most one sync wait
    per instruction.

    The stock _drain_and_barrier attaches one sem-wait per live proc to a
    single Drain instruction; this walrus build caps every non-EVSEM
    instruction at one sync wait and fails codegen ("Too many sync wait
    commands"). Emit one standalone single-wait instruction per semaphore
    instead, then a wait-less drain.
    """

    def _drain_and_barrier(self, tick_clock, wait_clock):
        nc = self.nc
        gc = tick_clock.global_clock
        for proc, sem in sorted(self.sems.allocated().items()):
            tick = gc.peek_next(proc) - 1
            if tick > 0:
                nc.sync.wait_ge(sem, tick_to_sem(tick, proc))
        nc.sync.drain()
        nc.all_engine_barrier()
        popped = nc._tile_sem_poison_stack.pop()
        assert popped is self._sem_poison
        nc.clear_and_free_semaphores(list(self.sems.allocated().values()))
        nc.all_engine_barrier()

F32 = mybir.dt.float32
F32R = mybir.dt.float32r
AF = mybir.ActivationFunctionType

B, S, D, H, DK = 2, 2048, 1024, 16, 64
HPC = 4                      # heads per core
NCORES = 8
IB_N = 4                     # i-blocks of 512
TILE_OFF = [0, 4, 12, 24]    # running tile index per i-block
NTILES = 40                  # total lower-triangle [128,512] tiles per head

_nc_cache = None
last_results = None          # stashed BassKernelResults (for test harness)


def build_nc():
    nc = bacc.Bacc("TRN2", target_bir_lowering=False)

    qT = nc.declare_dram_parameter("qT", [D, S], F32R, isOutput=False)
    kT = nc.declare_dram_parameter("kT", [D, S], F32R, isOutput=False)
    vT = nc.declare_dram_parameter("vT", [D, S], F32R, isOutput=False)
    wq = nc.declare_dram_parameter("wq", [D, 256], F32R, isOutput=False)
    wk = nc.declare_dram_parameter("wk", [D, 256], F32R, isOutput=False)
    wv = nc.declare_dram_parameter("wv", [D, 256], F32R, isOutput=False)
    bq = nc.declare_dram_parameter("bq", [256], F32, isOutput=False)
    bk = nc.declare_dram_parameter("bk", [256], F32, isOutput=False)
    bv = nc.declare_dram_parameter("bv", [256], F32R, isOutput=False)
    wo = nc.declare_dram_parameter("wo", [64, HPC, D], F32R, isOutput=False)
    masks = nc.declare_dram_parameter("masks", [128, 896], F32R, isOutput=False)
    ones_mat = nc.declare_dram_parameter("ones_mat", [128, 128], F32R, isOutput=False)

    attn_o = nc.declare_dram_parameter("attn_o", [HPC, NTILES, 128, 512], F32R, isOutput=True)
    outp = nc.declare_dram_parameter("outp", [S, D], F32, isOutput=True)
    sums_o = nc.declare_dram_parameter("sums_o", [HPC, S], F32R, isOutput=True)

    with TileContextSplitDrain(nc) as tc:
        with tc.tile_pool(name="const", bufs=1) as const:
            masks_sb = const.tile([128, 896], F32R, tag="masks")
            nc.sync.dma_start(out=masks_sb, in_=masks[:, :])
            wo_sb = const.tile([64, HPC, D], F32R, tag="wo")
            nc.sync.dma_start(out=wo_sb[:, :, :], in_=wo[:, :, :])
            bq_sb = const.tile([128, 2], F32, tag="bq")
            nc.sync.dma_start(out=bq_sb, in_=bq[:].rearrange("(c p) -> p c", p=128))
            bk_sb = const.tile([128, 2], F32, tag="bk")
            nc.sync.dma_start(out=bk_sb, in_=bk[:].rearrange("(c p) -> p c", p=128))
            bv_row = const.tile([1, 256], F32R, tag="bv")
            nc.sync.dma_start(out=bv_row, in_=bv[:].rearrange("(a f) -> a f", a=1))
            # ones constants come from DRAM: memset cannot write f32r
            ones_r = const.tile([65, 64], F32R, tag="ones_r")  # row 64 used (base-64 matmul operand)
            nc.sync.dma_start(out=ones_r[64:65, :], in_=ones_mat[0:1, 0:64])
            ones_c = const.tile([1, 128], F32R, tag="ones_c")
            nc.sync.dma_start(out=ones_c, in_=ones_mat[0:1, :])

            # persistent activations
            qhT_sb = const.tile([128, 2, S], F32R, tag="qhT")   # [d' (2 chunks), s]
            khT_sb = const.tile([128, 2, S], F32R, tag="khT")
            vh1_sb = const.tile([128, 16, HPC, 65], F32R, tag="vh1")  # [s, jc, h, dk+1]
            # col 64 = 1.0 (softmax-denominator column); cols 0:64 written by V proj
            nc.sync.dma_start(
                out=vh1_sb[:, :, :, 64:65],
                in_=ones_mat[:, 0:64].rearrange("p (a b c) -> p a b c", a=16, b=HPC),
            )

            # ---------------- projections ----------------
            with tc.tile_pool(name="psproj", bufs=8, space="PSUM") as psproj, \
                 tc.tile_pool(name="xin", bufs=3) as xin, \
                 tc.tile_pool(name="wts", bufs=2) as wts:

                for xext, wext, dst, bias_sb in (
                    (qT, wq, qhT_sb, bq_sb),
                    (kT, wk, khT_sb, bk_sb),
                ):
                    w_sb = wts.tile([128, 8, 256], F32R, tag="w")
                    nc.sync.dma_start(out=w_sb, in_=wext[:, :].rearrange("(kc p) m -> p kc m", p=128))
                    # two half-tensor loads (4 d-chunks each) instead of 8 small ones
                    xh = []
                    for half in range(2):
                        xt = xin.tile([128, 4, S], F32R, tag="x", name=f"xt{half}")
                        eng = nc.sync if half == 0 else nc.scalar
                        eng.dma_start(
                            out=xt,
                            in_=xext[half * 512:(half + 1) * 512, :].rearrange(
                                "(kc p) s -> p kc s", p=128),
                        )
                        xh.append(xt)
                    ps = [psproj.tile([128, 512], F32, tag="pp", name=f"ps{i}") for i in range(8)]
                    for kc in range(8):
                        xt = xh[kc // 4]
                        for mc in range(2):
                            for sb4 in range(4):
                                nc.tensor.matmul(
                                    ps[mc * 4 + sb4][:, :],
                                    lhsT=w_sb[:, kc, mc * 128:(mc + 1) * 128],
                                    rhs=xt[:, kc % 4, sb4 * 512:(sb4 + 1) * 512],
                                    start=(kc == 0), stop=(kc == 7),
                                )
                    for mc in range(2):
                        for sb4 in range(4):
                            nc.vector.tensor_scalar_add(
                                out=dst[:, mc, sb4 * 512:(sb4 + 1) * 512],
                                in0=ps[mc * 4 + sb4][:, :],
                                scalar1=bias_sb[:, mc:mc + 1],
                            )

                # V projection: vh[s, d'] per s-chunk; two sweeps of 8 s-chunks
                # (PSUM can only hold 8 accumulators); vT stays resident.
                w_sb = wts.tile([128, 8, 256], F32R, tag="w")
                nc.sync.dma_start(out=w_sb, in_=wv[:, :].rearrange("(kc p) m -> p kc m", p=128))
                vh = []
                for half in range(2):
                    xt = xin.tile([128, 4, S], F32R, tag="x", name=f"vt{half}")
                    eng = nc.sync if half == 0 else nc.scalar
                    eng.dma_start(
                        out=xt,
                        in_=vT[half * 512:(half + 1) * 512, :].rearrange(
                            "(kc p) s -> p kc s", p=128),
                    )
                    vh.append(xt)
                for sweep in range(2):
                    psv = [psproj.tile([128, 256], F32, tag="pp", name=f"psv{i}") for i in range(8)]
                    for kc in range(8):
                        xt = vh[kc // 4]
                        for j8 in range(8):
                            jc = sweep * 8 + j8
                            nc.tensor.matmul(
                                psv[j8][:, :],
                                lhsT=xt[:, kc % 4, jc * 128:(jc + 1) * 128],
                                rhs=w_sb[:, kc, :],
                                start=(kc == 0), stop=False,
                            )
                    for j8 in range(8):
                        jc = sweep * 8 + j8
                        nc.tensor.matmul(   # + b_v (rank-1: ones x bv)
                            psv[j8][:, :],
                            lhsT=ones_c, rhs=bv_row,
                            start=False, stop=True,
                        )
                        nc.vector.tensor_copy(
                            out=vh1_sb[:, jc, :, 0:64],
                            in_=psv[j8][:, :].rearrange("p (h d) -> p h d", h=HPC),
                        )

            # ---------------- attention + output projection ----------------
            with tc.tile_pool(name="pss", bufs=2, space="PSUM") as pss, \
                 tc.tile_pool(name="pso", bufs=2, space="PSUM") as pso, \
                 tc.tile_pool(name="psb", bufs=1, space="PSUM") as psb, \
                 tc.tile_pool(name="psp", bufs=1, space="PSUM") as psp, \
                 tc.tile_pool(name="expp", bufs=2) as expp, \
                 tc.tile_pool(name="outT", bufs=6) as outTp, \
                 tc.tile_pool(name="rbp", bufs=2) as rbp, \
                 tc.tile_pool(name="rcp", bufs=2) as rcp, \
                 tc.tile_pool(name="outs", bufs=2) as outs:

                for ib in range(IB_N):
                    jn = 4 * (ib + 1)
                    ot_tiles = []
                    for h in range(HPC):
                        po = (h % 2) * 64
                        mc = h // 2
                        pso_t = pso.tile([65, 512], F32, tag="po")
                        expg = expp.tile([128, jn, 512], F32R, tag="e")
                        for jp in range(jn // 2):
                            ps2 = pss.tile([128, 2, 512], F32, tag="ps")
                            for half in range(2):
                                jc = 2 * jp + half
                                nc.tensor.matmul(
                                    ps2[:, half, :],
                                    lhsT=khT_sb[po:po + 64, mc, jc * 128:(jc + 1) * 128],
                                    rhs=qhT_sb[po:po + 64, mc, ib * 512:(ib + 1) * 512],
                                    start=True, stop=True,
                                )
                            # one exp over both score tiles (halves ACT op count)
                            nc.scalar.activation(out=expg[:, 2 * jp:2 * jp + 2, :],
                                                 in_=ps2[:, :, :], func=AF.Exp, scale=0.125)
                            for half in range(2):
                                jc = 2 * jp + half
                                et = expg[:, jc, :]
                                if jc >= 4 * ib:
                                    off = 384 - (jc - 4 * ib) * 128
                                    nc.vector.tensor_mul(out=et, in0=et,
                                                         in1=masks_sb[:, off:off + 512])
                                nc.tensor.matmul(
                                    pso_t[:, :],
                                    lhsT=vh1_sb[:, jc, h, :],
                                    rhs=et,
                                    start=(jc == 0), stop=(jc == jn - 1),
                                )
                        # full tiles (incl. the d=0 diagonal tile) in one DMA;
                        # the last 3 diagonal tiles are column-trimmed: their
                        # leading d columns are always-masked zeros and the
                        # output buffer is zero-initialized, so skip them.
                        nc.sync.dma_start(
                            out=attn_o[h, TILE_OFF[ib]:TILE_OFF[ib] + jn - 3].rearrange(
                                "t p f -> p t f"),
                            in_=expg[:, 0:jn - 3, :],
                        )
                        for jc in range(jn - 3, jn):
                            d = (jc - 4 * ib) * 128
                            nc.sync.dma_start(
                                out=attn_o[h, TILE_OFF[ib] + jc, :, d:512],
                                in_=expg[:, jc, d:512],
                            )
                        # softmax denominators (row 64 of pso_t): reciprocal -> DRAM
                        rc_t = rcp.tile([65, 512], F32R, tag="rc")
                        with nc.allow_low_precision(reason="f32r recip feeds f32r matmuls"):
                            nc.vector.reciprocal(rc_t[64:65, :], pso_t[64:65, :])
                        nc.gpsimd.dma_start(
                            out=sums_o[h, ib * 512:(ib + 1) * 512].rearrange("(a f) -> a f", a=1),
                            in_=rc_t[64:65, :],
                        )
                        # broadcast 1/sum over the 64 dk partitions (rank-1 matmul)
                        psb_t = psb.tile([64, 512], F32, tag="pb")
                        nc.tensor.matmul(psb_t[:, :], lhsT=ones_r[64:65, :],
                                         rhs=rc_t[64:65, :], start=True, stop=True)
                        rb_t = rbp.tile([64, 512], F32, tag="rb")
                        nc.vector.tensor_copy(out=rb_t[:, :], in_=psb_t[:, :])
                        ot_t = outTp.tile([64, 512], F32R, tag="ot")
                        nc.vector.tensor_mul(out=ot_t[:, :], in0=pso_t[0:64, :], in1=rb_t[:, :])
                        ot_tiles.append(ot_t)

                    # w_o partial projection for this i-block
                    for ic in range(4):
                        i0 = ic * 128
                        ob_t = outs.tile([128, D], F32, tag="ob")
                        for nb in range(2):
                            pp_t = psp.tile([128, 512], F32, tag="ppj")
                            for h in range(HPC):
                                nc.tensor.matmul(
                                    pp_t[:, :],
                                    lhsT=ot_tiles[h][:, i0:i0 + 128],
                                    rhs=wo_sb[:, h, nb * 512:(nb + 1) * 512],
                                    start=(h == 0), stop=(h == HPC - 1),
                                )
                            nc.scalar.activation(out=ob_t[:, nb * 512:(nb + 1) * 512],
                                                 in_=pp_t[:, :], func=AF.Copy)
                        r0 = (ib * 4 + ic) * 128
                        nc.scalar.dma_start(out=outp[r0:r0 + 128, :], in_=ob_t)

    nc.compile()  # bacc: register allocation, DCE, event-sem legalization
    return nc


def _get_nc():
    global _nc_cache
    if _nc_cache is None:
        _nc_cache = build_nc()
    return _nc_cache


def make_core_inputs(q, k, v, w_q, b_q, w_k, b_k, w_v, b_v, w_o, c):
    b, hg = c // 4, c % 4
    sl = slice(hg * 256, hg * 256 + 256)
    masks = (np.arange(896)[None, :] >= (np.arange(128)[:, None] + 384)).astype(np.float32)
    return {
        "qT": np.ascontiguousarray(q[b].T),
        "kT": np.ascontiguousarray(k[b].T),
        "vT": np.ascontiguousarray(v[b].T),
        "wq": np.ascontiguousarray(w_q[sl, :].T),
        "wk": np.ascontiguousarray(w_k[sl, :].T),
        "wv": np.ascontiguousarray(w_v[sl, :].T),
        "bq": np.ascontiguousarray(b_q[sl]),
        "bk": np.ascontiguousarray(b_k[sl]),
        "bv": np.ascontiguousarray(b_v[sl]),
        "wo": np.ascontiguousarray(w_o[:, sl].T.reshape(HPC, 64, D).transpose(1, 0, 2)),
        "masks": masks,
        "ones_mat": np.ones((128, 128), np.float32),
    }


def gather(results, b_o):
    out = np.zeros((B, S, D), np.float32)
    attn = np.zeros((B, H, S, S), np.float32)
    for c in range(NCORES):
        r = results[c]
        b, hg = c // 4, c % 4
        out[b] += np.asarray(r["outp"])
        recips = np.asarray(r["sums_o"])     # [4, S] — 1/denominator
        ap = np.asarray(r["attn_o"])         # [4, 40, 128, 512]
        for h4 in range(HPC):
            h = hg * HPC + h4
            for ib in range(IB_N):
                jn = 4 * (ib + 1)
                blk = ap[h4, TILE_OFF[ib]:TILE_OFF[ib] + jn]      # [jc, j', i']
                s = recips[h4, ib * 512:(ib + 1) * 512]
                m = blk.transpose(2, 0, 1).reshape(512, jn * 128) * s[:, None]
                attn[b, h, ib * 512:(ib + 1) * 512, :jn * 128] = m
    out += np.asarray(b_o, np.float32)[None, None, :]
    return out, attn


def kernel(**inputs):
    global last_results
    q = np.asarray(inputs["q"], np.float32)
    k = np.asarray(inputs["k"], np.float32)
    v = np.asarray(inputs["v"], np.float32)
    w_q = np.asarray(inputs["w_q"], np.float32)
    b_q = np.asarray(inputs["b_q"], np.float32)
    w_k = np.asarray(inputs["w_k"], np.float32)
    b_k = np.asarray(inputs["b_k"], np.float32)
    w_v = np.asarray(inputs["w_v"], np.float32)
    b_v = np.asarray(inputs["b_v"], np.float32)
    w_o = np.asarray(inputs["w_o"], np.float32)
    b_o = np.asarray(inputs["b_o"], np.float32)

    nc = _get_nc()
    in_maps = [
        make_core_inputs(q, k, v, w_q, b_q, w_k, b_k, w_v, b_v, w_o, c)
        for c in range(NCORES)
    ]
    res = run_bass_kernel_spmd(nc, in_maps, list(range(NCORES)))
    last_results = res
    return gather(res.results, b_o)
